# revision 4
# baseline (speedup 1.0000x reference)
"""ArcFace combined-margin loss kernel for 8 TRN2 NeuronCores.

Strategy (v2)
-------------
reference: cos = (f @ w.T) / (|f||w|); phi = arcface(cos);
outputs = s*(labels*phi + (1-labels)*cos); loss = mean over rows of
-(sum of log_softmax(outputs) at lab_pinds, masked) / L^2.

The only device-scale work is the dense denominator
sexp[b] = sum_c exp(30*cos[b,c] - 30): per core B*C/8 fp8 MACs (PE,
~33.4us at 157 TF/s DoubleRow) and B*C/8 exps.  Everything else is
O(B*L*D + C*D) on host float64.

Per element the PE needs 0.833ns (2 K=256 passes); ACT exp is also
0.833ns/elem but its per-instruction overheads (352cyc issue + 187ns
accumulator read) push it ~5% over the PE rate, and any second
elementwise pass (row sums) would blow the budget.  So evictions are
split per 128-row block (16 blocks/core, 5 chunks of 500 classes):

  * 3 chunks -> one ACT Exp instruction with accum_out: the scalar
    engine's internal accumulator yields the row-sum of its share for
    free (sexp[:, block]), output itself is discarded into a scratch
    strip.  ACT/block = (1500+352)/1.2 + 187 ~ 1.73us < 2.08us PE.
  * 2 chunks -> DVE Schraudolph exp: one tensor_scalar computes
    bits = int16(psum*A + B), the bf16 bit pattern of ~exp(arg)
    (A,B fold the /16 scale, -30 bias, log2(e), the exponent bias and
    a mean-error-zeroing offset).  Strips DMA to HBM; the host sums
    them in f64 (elem err ~2% rms washes out over ~500-elem sums;
    validated loss rel err ~9e-5).  DVE/block ~ 1.34us.

PSUM is managed as one resident [128, 8, 512] tensor; chunk s in the
emission stream writes bank s%8, so each block's 5 chunks take 5
rotating banks and each ACT group of 3 is a constant-stride AP.
Blocks 0,1 are emitted chunk-interleaved so wT chunk c is first needed
~2c slots in (matches the HBM arrival order); block-major after.
Dummy fp8 matmuls warm the PE's DVFS p-state during the DMA lead-in.
Host (numpy float64): positive dots f.w[pinds] exactly, arcface margin,
denominator correction at positives, logsumexp, masked ragged CE, mean.
No collectives (8 partial sexp sets summed on host during unsharding).
"""

import math
import sys

import numpy as np
import ml_dtypes

for _p in ("/opt/trn_rl_repo",):
    if _p not in sys.path:
        sys.path.append(_p)

import concourse.bacc as bacc
import concourse.mybir as mybir
import concourse.tile as tile
from concourse.bass_utils import run_bass_kernel_spmd
from contextlib import ExitStack

B, C, D, LMAX = 2048, 20000, 512, 8
NCORES = 8
CSH = C // NCORES          # 2500 real classes per core
CW = 500                   # chunk width (5 chunks/block, fits one PSUM bank)
NCH = 5                    # chunks per block
NBLK = B // 128            # 16 row blocks
KC = D // 128              # 4 contraction chunks (128 partitions each)
S = 30.0
M_MARGIN = 0.5
FSC = 30.0                 # f rows scaled to 30*unit
WSC = 16.0                 # w rows scaled to 16*unit
# psum dot = FSC*WSC*cos; exp arg = psum/16 - 30 = 30*cos - 30

# Schraudolph bf16-bit exp: bits16 = trunc(psum*A_SCH + B_SCH) is the bf16
# bit pattern of ~exp(psum/16 - 30).  C offset 7.218 zeroes the mean of the
# linear-mantissa error over a uniform fraction; +0.5 centers truncation.
A_SCH = 128.0 * math.log2(math.e) / WSC
B_SCH = 128.0 * (127.0 - S * math.log2(math.e)) - 7.218 + 0.5

F32 = mybir.dt.float32
BF16 = mybir.dt.bfloat16
I16 = mybir.dt.int16
FP8 = mybir.dt.float8e4
E4M3 = ml_dtypes.float8_e4m3

_GRAPH = None


def _schedule():
    """Per-block chunk->bank map + ACT chunk set.

    Emission stream: blocks 0,1 chunk-interleaved, then block-major.
    Chunk at stream slot s lands in PSUM bank s%8.  Returns
    (emit_order, banks, act_chunks): emit_order = [(block, chunk)],
    banks[i][c] = bank, act_chunks[i] = tuple of 3 chunk ids whose banks
    form a constant-stride AP (the ACT accum group; other 2 go to DVE).
    """
    emit = []
    for c in range(NCH):
        emit.append((0, c))
        emit.append((1, c))
    for i in range(2, NBLK):
        for c in range(NCH):
            emit.append((i, c))
    banks = {}
    for s, (i, c) in enumerate(emit):
        banks.setdefault(i, {})[c] = s % 8
    act_chunks = {}
    for i in range(NBLK):
        if i < 2:
            act_chunks[i] = (0, 1, 2)      # banks stride 2: i, i+2, i+4
        else:
            off = banks[i][0]
            if off == 6:
                act_chunks[i] = (2, 3, 4)  # banks 0,1,2
            elif off == 7:
                act_chunks[i] = (1, 2, 3)  # banks 0,1,2
            else:
                act_chunks[i] = (0, 1, 2)  # banks off..off+2
    return emit, banks, act_chunks


def build_graph():
    nc = bacc.Bacc()
    # host supplies operands already in SBUF layout: [p, k, col] with
    # element (p, k, c) = x[c, k*128+p], so one DMA covers all k-chunks
    fT_ext = nc.declare_dram_parameter("fT8", [128, KC, B], FP8, isOutput=False)
    wT_ext = nc.declare_dram_parameter("wT8", [128, KC, CSH], FP8, isOutput=False)
    sexp_ext = nc.declare_dram_parameter("sexp", [128, NBLK], F32, isOutput=True)
    strips_ext = nc.declare_dram_parameter(
        "strips", [NBLK, 128, 1024], I16, isOutput=True
    )

    AF = mybir.ActivationFunctionType
    emit, banks, act_chunks = _schedule()

    with ExitStack() as ctx:
        tc = ctx.enter_context(tile.TileContext(nc))
        const = ctx.enter_context(tc.tile_pool(name="const", bufs=1))
        resident = ctx.enter_context(tc.tile_pool(name="resident", bufs=1))
        pmm = ctx.enter_context(tc.tile_pool(name="pmm", bufs=1, space="PSUM"))
        scr = ctx.enter_context(tc.tile_pool(name="scr", bufs=2))

        nbias = const.tile([128, 1], F32)
        nc.vector.memset(nbias[:], -S)
        dact = const.tile([128, 1], BF16)

        fT = resident.tile([128, KC, B], FP8)
        wT = resident.tile([128, KC, CSH], FP8)
        strip = resident.tile([128, NBLK, 1024], I16)
        sexp_t = resident.tile([128, NBLK], F32)
        psum = pmm.tile([128, 8, 512], F32)

        # Input DMA: per-queue issue costs ~600-800ns, so spread pieces
        # over the sync/gpsimd/vector queues ordered by first use.
        # fT0a (block-0 stationaries) + wTc0 unblock slot 0; wT chunk c
        # is first needed at stream slot 2c.
        nc.sync.dma_start(wT[:, :, 0:CW], wT_ext[:, :, 0:CW])
        nc.gpsimd.dma_start(fT[:, :, 0:128], fT_ext[:, :, 0:128])
        nc.scalar.dma_start(wT[:, :, CW : 2 * CW], wT_ext[:, :, CW : 2 * CW])
        nc.sync.dma_start(wT[:, :, 2 * CW : 3 * CW], wT_ext[:, :, 2 * CW : 3 * CW])
        nc.gpsimd.dma_start(fT[:, :, 128:512], fT_ext[:, :, 128:512])
        nc.gpsimd.dma_start(wT[:, :, 3 * CW : 4 * CW], wT_ext[:, :, 3 * CW : 4 * CW])
        nc.sync.dma_start(fT[:, :, 512:1024], fT_ext[:, :, 512:1024])
        nc.gpsimd.dma_start(fT[:, :, 1024:2048], fT_ext[:, :, 1024:2048])
        # preload the Exp activation table off the critical path; wT chunk 4
        # rides the scalar queue after it (needed ~slot 8)
        nc.scalar.activation(dact[:], nbias[:], AF.Exp, bias=nbias[:], scale=1.0)
        nc.scalar.dma_start(wT[:, :, 4 * CW : 5 * CW], wT_ext[:, :, 4 * CW : 5 * CW])

        # warm up the PE while the input DMAs land: the tensor engine's
        # clock p-state ramps with sustained use (0.65 -> 1.2 -> 2.4 GHz).
        # Warmups write banks 5-7, whose first real use is stream slot 5.
        warm = const.tile([128, 2, 512], FP8)
        nc.gpsimd.memset(warm[:], 0.0)
        for i in range(6):
            nc.tensor.matmul(
                psum[:, 5 + i % 3, :],
                warm[:, :, 0:128],
                warm[:],
                start=True,
                stop=True,
                perf_mode=mybir.MatmulPerfMode.DoubleRow,
            )

        # main stream: 2 DoubleRow matmuls (K=256 each) per chunk into its
        # bank; evictions fire as soon as their chunks complete.
        done_cnt = {i: 0 for i in range(NBLK)}
        dve_done = {i: 0 for i in range(NBLK)}
        nstrip_dma = 0
        for s, (i, c) in enumerate(emit):
            b = banks[i][c]
            for k2 in range(KC // 2):
                nc.tensor.matmul(
                    psum[:, b, 0:CW],
                    fT[:, 2 * k2 : 2 * k2 + 2, i * 128 : (i + 1) * 128],
                    wT[:, 2 * k2 : 2 * k2 + 2, c * CW : (c + 1) * CW],
                    start=(k2 == 0),
                    stop=(k2 == KC // 2 - 1),
                    perf_mode=mybir.MatmulPerfMode.DoubleRow,
                )
            done_cnt[i] += 1
            ac = act_chunks[i]
            if c not in ac:
                # DVE Schraudolph eviction straight into the strip tile
                j = dve_done[i]
                nc.vector.tensor_scalar(
                    strip[:, i, 512 * j : 512 * j + CW],
                    psum[:, b, 0:CW],
                    A_SCH,
                    B_SCH,
                    op0=mybir.AluOpType.mult,
                    op1=mybir.AluOpType.add,
                )
                dve_done[i] += 1
                if dve_done[i] == NCH - 3:
                    q = nc.gpsimd if (nstrip_dma % 2 == 0) else nc.sync
                    nstrip_dma += 1
                    q.dma_start(strips_ext[i], strip[:, i, :])
            # fire the ACT group once its last chunk is done (chunks are
            # emitted in increasing c order, so ac is complete at c==ac[-1])
            if c == ac[-1]:
                bs = [banks[i][x] for x in ac]
                st = bs[1] - bs[0]
                sc = scr.tile([128, 3, CW], BF16, tag="scr")
                nc.scalar.activation(
                    sc[:],
                    psum[:, bs[0] : bs[-1] + 1 : st, 0:CW],
                    AF.Exp,
                    bias=nbias[:],
                    scale=1.0 / WSC,
                    accum_out=sexp_t[:, i : i + 1],
                )
        nc.scalar.dma_start(sexp_ext[:, :], sexp_t[:, :])

    nc.finalize()
    return nc


def _get_graph():
    global _GRAPH
    if _GRAPH is None:
        _GRAPH = build_graph()
    return _GRAPH


def make_in_maps(f, lab_word2vec, lab_pinds=None):
    f = np.asarray(f, dtype=np.float32)
    w = np.asarray(lab_word2vec, dtype=np.float32)
    fn = np.sqrt((f.astype(np.float64) ** 2).sum(axis=1))
    wn = np.sqrt((w.astype(np.float64) ** 2).sum(axis=1))
    # [p, k, col] SBUF layout: element (p, k, c) = x[c, k*128+p]
    fT8 = np.ascontiguousarray(
        (f * (FSC / fn)[:, None].astype(np.float32)).T.astype(E4M3)
        .reshape(KC, 128, B).transpose(1, 0, 2)
    )
    w8 = (w * (WSC / wn)[:, None].astype(np.float32)).astype(E4M3)
    in_maps = []
    for i in range(NCORES):
        wc = w8[i * CSH : (i + 1) * CSH]
        wT8 = np.ascontiguousarray(
            wc.T.reshape(KC, 128, CSH).transpose(1, 0, 2)
        )
        in_maps.append({"fT8": fT8, "wT8": wT8})
    return in_maps


def combine(outs, f, lab_word2vec, lab_pinds, lengths):
    """outs: 8 dicts with sexp [128, NBLK] (ACT accums) and strips
    [NBLK, 128, 1024] int16 (bf16 bit patterns of the DVE share).
    Returns float32 loss."""
    f = np.asarray(f, dtype=np.float64)
    w = np.asarray(lab_word2vec, dtype=np.float64)
    pinds = np.asarray(lab_pinds, dtype=np.int64)
    lens = np.asarray(lengths, dtype=np.int64)

    # s_shift[b] = sum_c exp(30 cos - 30); b = i*128 + p
    s_shift = np.zeros(B, dtype=np.float64)
    for o in outs:
        per_block = o["sexp"].astype(np.float64)  # [128, NBLK]
        bits = np.asarray(o["strips"]).view(np.uint16).astype(np.uint32) << 16
        vals = bits.view(np.float32).astype(np.float64)  # [NBLK, 128, 1024]
        dve = vals[:, :, 0:CW].sum(axis=2) + vals[:, :, 512 : 512 + CW].sum(axis=2)
        s_shift += (per_block + dve.T).T.reshape(B)

    fn = np.sqrt((f * f).sum(axis=1))     # [B]
    wn = np.sqrt((w * w).sum(axis=1))     # [C]
    pd = np.einsum("bjd,bd->bj", w[pinds], f)              # [B, LMAX]
    cos = pd / np.maximum(fn[:, None] * wn[pinds], 1e-8)

    cos_m, sin_m = math.cos(M_MARGIN), math.sin(M_MARGIN)
    th = math.cos(math.pi - M_MARGIN)
    mm = math.sin(math.pi - M_MARGIN) * M_MARGIN
    sine = np.sqrt(np.clip(1.0 - cos * cos, 0.0, 1.0))
    phi = cos * cos_m - sine * sin_m
    phi = np.where(cos > th, phi, cos - mm)

    mask = (np.arange(LMAX)[None, :] < lens[:, None]).astype(np.float64)
    corr = (mask * (np.exp(S * phi - S) - np.exp(S * cos - S))).sum(axis=1)
    z = S + np.log(s_shift + corr)  # logsumexp of outputs, [B]
    pos_sum = (mask * (S * phi)).sum(axis=1)
    L = lens.astype(np.float64)
    per_sample = (L * z - pos_sum) / (L * L)
    return np.float32(per_sample.mean())


def kernel(f, labels, lab_word2vec, lab_pinds, lengths):
    nc = _get_graph()
    in_maps = make_in_maps(f, lab_word2vec)
    res = run_bass_kernel_spmd(nc, in_maps, core_ids=list(range(NCORES)))
    return combine(res.results, f, lab_word2vec, lab_pinds, lengths)
